# revision 37
# baseline (speedup 1.0000x reference)
"""Trainium2 Bass kernel for a dense transformer block (B=2, S=2048, D=2048,
H=16, head_dim=128, FF=8192, fp32 I/O), SPMD over 8 NeuronCores.

Sharding: data-parallel over tokens, batch-interleaved: core c owns tokens
[256c, 256c+256) of BOTH batches (512 tokens total). Attention needs all
keys/values of each batch, so K^T and V (fp8) are AllGather'd over all 8
cores — batch-interleaving keeps the gathered layout identical on every core
(no core-dependent addressing) and the 8-rank chip-wide AllGather is much
faster per byte than a 4-rank ring.

Layout: activations live feature-major ("transposed", [D, tokens]) on chip so
every GEMM contracts along the partition axis with weights in natural layout.
The host transposes x in / y out. LayerNorm stats and softmax denominators
are partition-axis reductions done with ones-matmuls on the PE.

dtypes: all six projection GEMMs run in fp8 e4m3 with DoubleRow perf mode
(2 contraction rows per PE pass): weights are pre-scaled by 2^13 and cast to
fp8 on the host (values ~1/sqrt(D) are subnormal in raw e4m3); activations
cast to fp8 at natural scale (LN outputs / gelu outputs are O(1)); the 2^-13
is folded into the post-PSUM activation's input scale. Attention O is ~1e-2
magnitude so it is scaled by 2^7 before the fp8 cast (o_proj result scaled by
2^-20). Scores matmuls are fp8 non-DoubleRow (contraction = head_dim = 128
only); exp outputs are fp8 and feed DoubleRow PV matmuls (pairs of 128-key
chunks). Both heads of a pair share one exp tile per key-group so the softmax
denominator is ONE 512-wide DoubleRow ones-matmul per group. LN stats and
residuals stay fp32; all matmul accumulation is fp32 in PSUM.

Paired layouts for DoubleRow: activations/weights are stored as [128, 2, N]
tiles where dim1 holds two consecutive 128-row contraction chunks. Weight /
K^T / V gather loads use single rearranged-AP DMAs ("(c i p) n -> c p i n")
— one descriptor per tile instead of per 128-row block; per-trigger queue
cost (~0.6us) made 32-descriptor vhp loads the attention bottleneck.

Scheduling notes (all measured on HW traces):
- weight streams use [128, 2, 1024] tiles with deep (bufs=10-12) prefetch;
  shallow prefetch stalls the PE on DMA latency and re-throttles the PE
  clock (HAM). Deepening 8->10/12 measured ~-25us; deeper still regresses
  (SBUF pressure).
- the first weight tiles of each GEMM phase are preloaded before the
  preceding LayerNorm / prior phase: a fresh pool's space reuses
  just-released scratch, so a late first-touch would chain the weight
  stream behind the previous phase's tail.
- AllGathers are ordered K0, V0, K1, V1 so attention pair 0 (needs
  kout0+vout0) unblocks after two collectives. All collectives issue from
  gpsimd (NRT needs a straight-line collective order); NOTHING else that
  later phases wait on may be issued on gpsimd after them, because each
  collective blocks that queue until it completes.
- K/V staging copies run on the DVE: the scalar engine still owns LN1's
  affine tail, and staging through it delayed the first AllGather ~20us.
- the attention exp (ACT) is software-pipelined 3 key-groups ahead of the
  PV/denominator matmuls; score groups are 2 key-chunks x 2 heads (one
  2-bank PSUM tile, ONE exp ACT per group — per-ACT fixed cost ~350 cycles
  made 4 small exps per group the attention bottleneck).
- LayerNorm is computed as E[x^2]-E[x]^2: squares+stat-matmuls pipeline
  with chunk production (x loads for LN1, o_proj residuals for LN2 — whose
  stats are woven into o_proj's four 4-chunk column blocks).
"""
import sys

for _p in (
    "/root/.axon_site",
    "/root/.axon_site/_ro/trn_rl_repo",
    "/root/.axon_site/_ro/pypackages",
):
    if _p not in sys.path:
        sys.path.insert(0, _p)

import ml_dtypes
import numpy as np

import concourse.bacc as bacc
import concourse.tile as tile
import concourse.mybir as mybir
from concourse import bass_utils
from concourse.alu_op_type import AluOpType
from concourse.bass_interp import get_hw_module

B, S, D = 2, 2048, 2048
H, HD, FF = 16, 128, 8192
N_CORES = 8
TB = S // N_CORES  # 256 tokens of each batch per core
T = B * TB  # 512 tokens per core (256 b0 + 256 b1)
NCH = D // 128  # 16 feature chunks
NPH = NCH // 2  # 8 feature-chunk pairs
FFCH = FF // 128  # 64 ff chunks
FFPH = FFCH // 2  # 32 ff-chunk pairs
F32 = mybir.dt.float32
F32R = mybir.dt.float32r
BF16 = mybir.dt.bfloat16
F8 = mybir.dt.float8e4
AF = mybir.ActivationFunctionType
OP = AluOpType
DR = mybir.MatmulPerfMode.DoubleRow
SM_SCALE = 1.0 / float(np.sqrt(HD))
SW = 8192.0  # weight fp8 prescale (2^13)
RSW = 1.0 / SW
SO = 128.0  # attention-out fp8 prescale (2^7)
WBUFS = 10  # weight-stream prefetch depth


def _wpair(wd, npair):
    """[N*256, M] weight dram tensor -> [npair, 128, 2, M] paired view."""
    return wd.ap().rearrange("(c i p) n -> c p i n", c=npair, i=2, p=128)


class _LN:
    """LayerNorm in E[x^2]-E[x]^2 form: the per-chunk squares and both stat
    matmuls have no dependency on the mean, so they pipeline with chunk
    production (x DMAs for LN1, o_proj residual writes for LN2) instead of
    serializing a second pass. mu << sigma here so there is no cancellation.
    Uses 2 PSUM banks; stats on ACT(square)+PE, normalize on DVE+ACT."""

    def __init__(self, nc, lp, ls, lps, ones_r, name):
        self.nc, self.lp, self.ls, self.ones_r = nc, lp, ls, ones_r
        self.mu_ps = lps.tile([128, T], F32, tag=f"{name}mu", name=f"{name}mu")
        self.var_ps = lps.tile([128, T], F32, tag=f"{name}var", name=f"{name}var")

    def stats_chunk(self, c, src_c):
        nc = self.nc
        sq = self.ls.tile([128, T], F32R, tag="sq")
        nc.scalar.activation(sq[:], src_c[:].bitcast(F32), AF.Square)
        nc.tensor.matmul(
            self.mu_ps[:], self.ones_r[:], src_c[:],
            start=(c == 0), stop=(c == NCH - 1),
        )
        nc.tensor.matmul(
            self.var_ps[:], self.ones_r[:], sq[:],
            start=(c == 0), stop=(c == NCH - 1),
        )

    def normalize(self, src, dst, g_s, b_s, eps_t):
        nc = self.nc
        # mean to SBUF via ACT (ScalarE has the fast PSUM port; DVE reading
        # PSUM per-chunk measured ~15us slower across both LNs)
        mu = self.lp.tile([128, T], F32, tag="mu")
        nc.scalar.activation(mu[:], self.mu_ps[:], AF.Copy, scale=1.0 / D)
        msq = self.lp.tile([128, T], F32, tag="msq")
        nc.vector.tensor_tensor(msq[:], mu[:], mu[:], OP.mult)
        # var = var_ps/D - mu^2
        var = self.lp.tile([128, T], F32, tag="var")
        nc.vector.scalar_tensor_tensor(
            var[:], self.var_ps[:], 1.0 / D, msq[:], OP.mult, OP.subtract
        )
        sd = self.lp.tile([128, T], F32, tag="sd")
        nc.scalar.activation(sd[:], var[:], AF.Sqrt, bias=eps_t[:])
        rsq = self.lp.tile([128, T], F32, tag="rsq")
        nc.vector.reciprocal_approx_fast(rsq[:], sd[:])

        for c in range(NCH):
            xc = self.ls.tile([128, T], F32, tag="xc")
            nc.vector.tensor_tensor(
                xc[:], src[c][:].bitcast(F32), mu[:], OP.subtract
            )
            tmp = self.ls.tile([128, T], F32, tag="lnt")
            nc.vector.tensor_tensor(tmp[:], xc[:], rsq[:], OP.mult)
            nc.scalar.activation(
                dst[c], tmp[:], AF.Identity,
                bias=b_s[:, c:c + 1], scale=g_s[:, c:c + 1],
            )


def build():
    nc = bacc.Bacc("TRN2", target_bir_lowering=False, debug=False,
                   num_devices=N_CORES)

    xT_d = nc.dram_tensor("xT", [D, T], F32, kind="ExternalInput")
    wq_d = nc.dram_tensor("wq8", [D, D], F8, kind="ExternalInput")
    wk_d = nc.dram_tensor("wk8", [D, D], F8, kind="ExternalInput")
    wv_d = nc.dram_tensor("wv8", [D, D], F8, kind="ExternalInput")
    wo_d = nc.dram_tensor("wo8", [D, D], F8, kind="ExternalInput")
    w1_d = nc.dram_tensor("w18", [D, FF], F8, kind="ExternalInput")
    w2_d = nc.dram_tensor("w28", [FF, D], F8, kind="ExternalInput")
    b1_d = nc.dram_tensor("b1r", [128, FFCH], F32, kind="ExternalInput")
    b2_d = nc.dram_tensor("b2r", [128, NCH], F32, kind="ExternalInput")
    g1_d = nc.dram_tensor("g1r", [128, NCH], F32, kind="ExternalInput")
    be1_d = nc.dram_tensor("be1r", [128, NCH], F32, kind="ExternalInput")
    g2_d = nc.dram_tensor("g2r", [128, NCH], F32, kind="ExternalInput")
    be2_d = nc.dram_tensor("be2r", [128, NCH], F32, kind="ExternalInput")
    yT_d = nc.dram_tensor("yT", [D, T], F32, kind="ExternalOutput")

    wqp_v = _wpair(wq_d, NPH)
    wkp_v = _wpair(wk_d, NPH)
    wvp_v = _wpair(wv_d, NPH)
    wop_v = _wpair(wo_d, NPH)
    w1p_v = _wpair(w1_d, NPH)
    w2p_v = _wpair(w2_d, FFPH)

    with tile.TileContext(nc) as tc:
        with (
            tc.tile_pool(name="cst", bufs=1) as cst,
            tc.tile_pool(name="resid", bufs=1) as resid,
            tc.tile_pool(name="dram", bufs=1, space="DRAM") as dram,
        ):
            ones_r = cst.tile([128, 128], F32R)
            nc.vector.memset(ones_r[:].bitcast(F32), 1.0)
            ones8p = cst.tile([128, 2, 128], F8)
            nc.vector.memset(ones8p[:], 1.0)
            eps_t = cst.tile([128, 1], F32)
            nc.vector.memset(eps_t[:], 1e-5)
            g1_s = cst.tile([128, NCH], F32)
            be1_s = cst.tile([128, NCH], F32)
            g2_s = cst.tile([128, NCH], F32)
            be2_s = cst.tile([128, NCH], F32)
            b1_s = cst.tile([128, FFCH], F32)
            b2_s = cst.tile([128, NCH], F32)
            nc.sync.dma_start(g1_s[:], g1_d.ap())
            nc.sync.dma_start(be1_s[:], be1_d.ap())
            nc.sync.dma_start(g2_s[:], g2_d.ap())
            nc.sync.dma_start(be2_s[:], be2_d.ap())
            nc.sync.dma_start(b1_s[:], b1_d.ap())
            nc.sync.dma_start(b2_s[:], b2_d.ap())

            # x^T resident, per-chunk tiles; loads split across two trigger
            # engines so the initial burst uses two DMA queues
            xTs = [resid.tile([128, T], F32R, name=f"xT{c}") for c in range(NCH)]
            for c in range(NCH):
                eng = (nc.sync, nc.gpsimd, nc.scalar)[c % 3]
                eng.dma_start(
                    xTs[c][:],
                    xT_d.ap()[c * 128:(c + 1) * 128, :].bitcast(F32R),
                )
            x2Ts = [resid.tile([128, T], F32R, name=f"x2T{c}") for c in range(NCH)]

            kin0 = dram.tile([D // 2, T], F8)
            kin1 = dram.tile([D // 2, T], F8)
            vin0 = dram.tile([T, D // 2], F8)
            vin1 = dram.tile([T, D // 2], F8)
            kout0 = dram.tile([N_CORES * D // 2, T], F8, addr_space="Shared")
            kout1 = dram.tile([N_CORES * D // 2, T], F8, addr_space="Shared")
            vout0 = dram.tile([N_CORES * T, D // 2], F8, addr_space="Shared")
            vout1 = dram.tile([N_CORES * T, D // 2], F8, addr_space="Shared")

            with (
                tc.tile_pool(name="attnres", bufs=1) as ares,
                tc.tile_pool(name="wopre", bufs=1) as wop,
                tc.tile_pool(name="ffnres", bufs=1) as fres,
                tc.tile_pool(name="w1pre", bufs=1) as w1p,
            ):
                # Q^T per head pair; slot reused for O^T after the pair is
                # done (dim1 = which head of the pair)
                qTp = [ares.tile([128, 2, T], F8, name=f"qTp{p}")
                       for p in range(H // 2)]
                # fc1 input (LN2 output) — allocated early so LN2/fc1 pools
                # outlive the attention block that produces x2
                h2T2 = [fres.tile([128, 2, T], F8, name=f"h2T2_{c}")
                        for c in range(NPH)]

                with tc.tile_pool(name="p1", bufs=1) as p1:
                    # preload first K-block weights BEFORE LN1 so the weight
                    # stream isn't serialized behind the LN scratch release
                    wk0 = [p1.tile([128, 2, 1024], F8, name=f"wk0_{c}")
                           for c in range(NPH)]
                    for c in range(NPH):
                        nc.sync.dma_start(wk0[c][:], wkp_v[c, :, :, 0:1024])
                    hT2 = [p1.tile([128, 2, T], F8, name=f"hT2_{c}")
                           for c in range(NPH)]
                    with (
                        tc.tile_pool(name="ln1_p", bufs=1) as l1p,
                        tc.tile_pool(name="ln1_s", bufs=3) as l1s,
                        tc.tile_pool(name="ln1_ps", bufs=1, space="PSUM") as l1ps,
                    ):
                        ln1 = _LN(nc, l1p, l1s, l1ps, ones_r, "ln1")
                        for c in range(NCH):
                            ln1.stats_chunk(c, xTs[c])
                        ln1.normalize(
                            xTs,
                            [hT2[c // 2][:, c % 2, :] for c in range(NCH)],
                            g1_s, be1_s, eps_t,
                        )

                    with (
                        tc.tile_pool(name="qkvs", bufs=WBUFS) as qs,
                        tc.tile_pool(name="qkvstg", bufs=6) as stg,
                        tc.tile_pool(name="wqpre", bufs=1) as wqp,
                        tc.tile_pool(name="qkvps", bufs=1, space="PSUM") as qps,
                    ):
                        # prefetch Q blk0 weights on the gpsimd queue BEFORE
                        # any collective is emitted there (collectives block
                        # the issuing engine until completion)
                        wq0 = []
                        for blk in range(2):
                            for c in range(NPH):
                                wt = wqp.tile([128, 2, 1024], F8,
                                              name=f"wq0pre_{blk}_{c}")
                                nc.gpsimd.dma_start(
                                    wt[:],
                                    wqp_v[c, :, :, blk * 1024:(blk + 1) * 1024],
                                )
                                wq0.append(wt)

                        def k_block(blk):
                            """K^T = wk.T @ h^T (feeds the AllGather)"""
                            kps = [qps.tile([128, T], F32, tag=f"qkv{q}", name=f"qkvps{q}")
                                   for q in range(8)]
                            for c in range(NPH):
                                if blk == 0:
                                    wt = wk0[c]
                                else:
                                    wt = qs.tile([128, 2, 1024], F8, tag="w")
                                    nc.sync.dma_start(
                                        wt[:],
                                        wkp_v[c, :, :,
                                              blk * 1024:(blk + 1) * 1024],
                                    )
                                for q in range(8):
                                    nc.tensor.matmul(
                                        kps[q][:],
                                        wt[:, :, q * 128:(q + 1) * 128],
                                        hT2[c][:],
                                        start=(c == 0), stop=(c == NPH - 1),
                                        perf_mode=DR,
                                    )
                            kin_h = kin0 if blk == 0 else kin1
                            for q in range(8):
                                # staging copy on DVE: the scalar engine is
                                # busy with LN1's affines and would delay the
                                # AllGather launch
                                ks = stg.tile([128, T], F8, tag="kstg")
                                nc.vector.tensor_scalar_mul(
                                    ks[:], kps[q][:], RSW
                                )
                                nc.sync.dma_start(
                                    kin_h[q * 128:(q + 1) * 128, :], ks[:]
                                )
                            nc.gpsimd.collective_compute(
                                "AllGather",
                                OP.bypass,
                                replica_groups=[list(range(N_CORES))],
                                ins=[(kin0 if blk == 0 else kin1).opt()],
                                outs=[(kout0 if blk == 0 else kout1).opt()],
                            )

                        def v_block(blk):
                            """V = h @ wv (natural layout: lhsT = h^T pair)"""
                            vps = [qps.tile([128, T], F32, tag=f"qkv{q}", name=f"qkvps{q}")
                                   for q in range(8)]
                            for c in range(NPH):
                                wt = qs.tile([128, 2, 1024], F8, tag="w")
                                nc.sync.dma_start(
                                    wt[:],
                                    wvp_v[c, :, :, blk * 1024:(blk + 1) * 1024],
                                )
                                for sub in range(2):
                                    for t_ in range(4):
                                        nc.tensor.matmul(
                                            vps[sub * 4 + t_][:],
                                            hT2[c][:, :, t_ * 128:(t_ + 1) * 128],
                                            wt[:, :, sub * 512:(sub + 1) * 512],
                                            start=(c == 0), stop=(c == NPH - 1),
                                            perf_mode=DR,
                                        )
                            vin_h = vin0 if blk == 0 else vin1
                            for sub in range(2):
                                for t_ in range(4):
                                    vs = stg.tile([128, 512], F8, tag="vstg")
                                    nc.vector.tensor_scalar_mul(
                                        vs[:], vps[sub * 4 + t_][:], RSW
                                    )
                                    nc.sync.dma_start(
                                        vin_h[t_ * 128:(t_ + 1) * 128,
                                              sub * 512:(sub + 1) * 512],
                                        vs[:],
                                    )
                            nc.gpsimd.collective_compute(
                                "AllGather",
                                OP.bypass,
                                replica_groups=[list(range(N_CORES))],
                                ins=[(vin0 if blk == 0 else vin1).opt()],
                                outs=[(vout0 if blk == 0 else vout1).opt()],
                            )

                        # K0 -> V0 -> K1 -> V1 so the first attention head
                        # pair (needs kout0+vout0) unblocks after TWO
                        # AllGathers instead of three
                        k_block(0)
                        v_block(0)
                        k_block(1)
                        v_block(1)

                        # Q^T (overlaps the collectives)
                        for blk in range(2):
                            qph = [qps.tile([128, T], F32, tag=f"qkv{q}", name=f"qkvps{q}")
                                   for q in range(8)]
                            for c in range(NPH):
                                wt = wq0[blk * NPH + c]
                                for q in range(8):
                                    nc.tensor.matmul(
                                        qph[q][:],
                                        wt[:, :, q * 128:(q + 1) * 128],
                                        hT2[c][:],
                                        start=(c == 0), stop=(c == NPH - 1),
                                        perf_mode=DR,
                                    )
                            for q in range(8):
                                hh = blk * 8 + q
                                nc.scalar.activation(
                                    qTp[hh // 2][:, hh % 2, :], qph[q][:],
                                    AF.Copy, scale=RSW,
                                )

                # attention: heads processed in interleaved pairs so the PE
                # always has one head's independent matmuls to run while the
                # other head's exp (ACT) is in flight. Keys of both batches:
                # 32 chunks of 128 per head, processed as 16 groups of 2
                # (1 PSUM bank per score group). Groups g<8: batch-0 keys
                # (query cols 0:256); g>=8: batch-1 (cols 256:512). Both
                # heads of a pair share one exp tile per group: cols 0:256 =
                # head A, 256:512 = head B, so the denominator is a single
                # 512-wide DoubleRow ones-matmul per group, accumulated into
                # per-batch psum [128, 2*TB] (head-major columns).
                with (
                    tc.tile_pool(name="atts", bufs=4) as ats,
                    tc.tile_pool(name="attv", bufs=3) as atv,
                    tc.tile_pool(name="attes", bufs=8) as aes,
                    tc.tile_pool(name="attrec", bufs=2) as arc,
                    tc.tile_pool(name="attps", bufs=2, space="PSUM") as aps,
                    tc.tile_pool(name="attps2", bufs=2, space="PSUM") as aps2,
                ):
                    # preload the first o_proj column block (all 8 head
                    # pairs) during attention
                    wo0 = [wop.tile([128, 2, 512], F8, name=f"wo0_{p}")
                           for p in range(H // 2)]
                    for p in range(H // 2):
                        nc.sync.dma_start(wo0[p][:], wop_v[p, :, :, 0:512])

                    def s_group(pair, g, kTs, exps):
                        """scores for both heads of the pair, then ONE exp
                        ACT over the shared [128, 2, 512] 2-bank score psum
                        — per-ACT fixed cost (~350 cyc) made 32 small exps
                        per pair the attention-phase bottleneck"""
                        bb = g // 8  # batch half
                        s_ps = aps.tile([128, 2, 2 * TB], F32, tag="s",
                                        name="s_ps")
                        for idx, hh in enumerate(pair):
                            for i in range(2):
                                kb = (g % 8) * 2 + i  # batch-local key chunk
                                r, half = kb // 2, kb % 2
                                nc.tensor.matmul(
                                    s_ps[:, i, idx * TB:(idx + 1) * TB],
                                    kTs[hh][:, r, bb * 256 + half * 128:
                                            bb * 256 + half * 128 + 128],
                                    qTp[hh // 2][:, hh % 2,
                                        bb * 256:(bb + 1) * 256],
                                    start=True, stop=True,
                                )
                        nc.scalar.activation(exps[:], s_ps[:], AF.Exp,
                                             scale=SM_SCALE)

                    def pvden_group(g, exps, vh, pvA, pvB, den2):
                        bb = g // 8
                        qsl = slice(bb * 256, (bb + 1) * 256)
                        r = g % 8  # rank owning this pair of key chunks
                        m0 = r * 4 + bb * 2  # row-block pair base in vout
                        first = (g % 8) == 0
                        last = (g % 8) == 7
                        for hpar, pv_ps in ((0, pvA), (1, pvB)):
                            nc.tensor.matmul(
                                pv_ps[:, qsl],
                                vh[:, m0:m0 + 2, hpar * 128:hpar * 128 + 128],
                                exps[:, :, hpar * TB:(hpar + 1) * TB],
                                start=first, stop=last,
                                perf_mode=DR,
                            )
                        nc.tensor.matmul(
                            den2[bb][:], ones8p[:], exps[:],
                            start=first, stop=last,
                            perf_mode=DR,
                        )

                    for hp in range(H // 2):
                        pair = (2 * hp, 2 * hp + 1)
                        kout_h = kout0 if pair[0] < 8 else kout1
                        vout_h = vout0 if pair[0] < 8 else vout1
                        kout_v = kout_h[:, :].rearrange(
                            "(r h p) c -> h p r c", r=N_CORES, h=8, p=128
                        )
                        vout_v = vout_h[:, :].rearrange(
                            "(m p) c -> p m c", m=32, p=128
                        )
                        hb = (hp % 4) * 256  # V column base within the half
                        kTs, pvs = {}, {}
                        for hh in pair:
                            kTs[hh] = ats.tile([128, N_CORES, T], F8, tag="kT", name="kT")
                            # two descriptors so the load stripes across two
                            # DMA engines (one big descriptor runs on a
                            # single engine at ~1/16th of HBM bandwidth)
                            for rh in range(2):
                                nc.sync.dma_start(
                                    kTs[hh][:, rh * 4:(rh + 1) * 4, :],
                                    kout_v[hh % 8, :, rh * 4:(rh + 1) * 4, :],
                                )
                            pvs[hh] = aps2.tile([128, T], F32, tag="pv", name="pv_ps")
                        den2 = [aps2.tile([128, 2 * TB], F32, tag="den",
                                          name="den_ps") for _ in range(2)]
                        # V columns for BOTH heads of the pair in one tile,
                        # four rearranged-AP descriptors. NOT on gpsimd: the
                        # collectives block that queue until they complete,
                        # which would stall every pair's PV on the last
                        # AllGather (only gpsimd/sync/scalar can trigger DMA)
                        vhp = atv.tile([128, 32, 2 * HD], F8, tag="vh", name="vh")
                        for mq in range(4):
                            nc.sync.dma_start(
                                vhp[:, mq * 8:(mq + 1) * 8, :],
                                vout_v[:, mq * 8:(mq + 1) * 8, hb:hb + 256],
                            )
                        # interleaved stream of key-groups with PV/den
                        # trailing 2 groups behind the score/exp
                        pending = []
                        for g in range(16):
                            exps = aes.tile([128, 2, 2 * TB], F8, tag="exp",
                                            name="exps")
                            s_group(pair, g, kTs, exps)
                            pending.append((g, exps))
                            if len(pending) > 3:
                                pg, pe = pending.pop(0)
                                pvden_group(pg, pe, vhp, pvs[pair[0]],
                                            pvs[pair[1]], den2)
                        for pg, pe in pending:
                            pvden_group(pg, pe, vhp, pvs[pair[0]],
                                        pvs[pair[1]], den2)

                        for hh in pair:
                            ho = (hh % 2) * TB
                            rec = arc.tile([128, T], F32, tag="rec", name="rec")
                            for bb in range(2):
                                nc.vector.reciprocal_approx_fast(
                                    rec[:, bb * TB:(bb + 1) * TB],
                                    den2[bb][:, ho:ho + TB],
                                )
                            # overwrite Q^T slot with O^T (Q^T[hh] is dead
                            # now); O is ~1e-2 magnitude -> prescale by 2^7
                            nc.vector.scalar_tensor_tensor(
                                qTp[hp][:, hh % 2, :], pvs[hh][:], SO, rec[:],
                                OP.mult, OP.mult,
                            )

                # o_proj + residual -> x2T, in four 4-chunk column blocks so
                # LN2's stats (2 PSUM banks) run woven between them: each
                # chunk's square + stat matmuls issue right after its
                # residual write, overlapping the remaining o_proj work
                # preload first fc1 weight block; streams during o_proj
                w10 = [w1p.tile([128, 2, 1024], F8, name=f"w10_{c}")
                      for c in range(NPH)]
                for c in range(NPH):
                    nc.sync.dma_start(w10[c][:], w1p_v[c, :, :, 0:1024])
                with (
                    tc.tile_pool(name="ops", bufs=12) as osp,
                    tc.tile_pool(name="opps", bufs=1, space="PSUM") as ops_ps,
                    tc.tile_pool(name="ln2_p", bufs=1) as l2p,
                    tc.tile_pool(name="ln2_s", bufs=3) as l2s,
                    tc.tile_pool(name="ln2_ps", bufs=1, space="PSUM") as l2ps,
                ):
                    ln2 = _LN(nc, l2p, l2s, l2ps, ones_r, "ln2")
                    for cb in range(4):
                        o_ps = [ops_ps.tile([128, T], F32, tag=f"o{q}", name=f"ops{q}")
                                for q in range(4)]
                        for p in range(H // 2):
                            if cb == 0:
                                wt = wo0[p]
                            else:
                                wt = osp.tile([128, 2, 512], F8, tag="wo")
                                nc.sync.dma_start(
                                    wt[:],
                                    wop_v[p, :, :, cb * 512:(cb + 1) * 512],
                                )
                            for q in range(4):
                                nc.tensor.matmul(
                                    o_ps[q][:],
                                    wt[:, :, q * 128:(q + 1) * 128],
                                    qTp[p][:],
                                    start=(p == 0), stop=(p == H // 2 - 1),
                                    perf_mode=DR,
                                )
                        for q in range(4):
                            dc = cb * 4 + q
                            nc.vector.scalar_tensor_tensor(
                                x2Ts[dc][:], o_ps[q][:],
                                RSW / SO, xTs[dc][:].bitcast(F32),
                                OP.mult, OP.add,
                            )
                            ln2.stats_chunk(dc, x2Ts[dc])
                    ln2.normalize(
                        x2Ts,
                        [h2T2[c // 2][:, c % 2, :] for c in range(NCH)],
                        g2_s, be2_s, eps_t,
                    )

                # FFN
                with tc.tile_pool(name="gpool", bufs=1) as gp:
                    gres2 = [gp.tile([128, 2, T], F8, name=f"g2_{f}")
                             for f in range(FFPH)]
                    # preload the first fc2 tiles during fc1 so the fc2
                    # weight stream isn't cold at the phase switch
                    w20 = [gp.tile([128, 2, 1024], F8, name=f"w20_{f}")
                           for f in range(8)]
                    for f in range(8):
                        nc.scalar.dma_start(w20[f][:], w2p_v[f, :, :, 0:1024])
                    with (
                        tc.tile_pool(name="fc1s", bufs=12) as fs1,
                        tc.tile_pool(name="fc1ps", bufs=1, space="PSUM") as f1ps,
                    ):
                        for fb in range(8):
                            a_ps = [f1ps.tile([128, T], F32, tag=f"a{q}", name=f"aps{q}")
                                    for q in range(8)]
                            for c in range(NPH):
                                if fb == 0:
                                    wt = w10[c]
                                else:
                                    wt = fs1.tile([128, 2, 1024], F8, tag="w1")
                                    nc.sync.dma_start(
                                        wt[:],
                                        w1p_v[c, :, :,
                                              fb * 1024:(fb + 1) * 1024],
                                    )
                                for q in range(8):
                                    nc.tensor.matmul(
                                        a_ps[q][:],
                                        wt[:, :, q * 128:(q + 1) * 128],
                                        h2T2[c][:],
                                        start=(c == 0), stop=(c == NPH - 1),
                                        perf_mode=DR,
                                    )
                            for q in range(8):
                                ffc = fb * 8 + q
                                nc.scalar.activation(
                                    gres2[ffc // 2][:, ffc % 2, :], a_ps[q][:],
                                    AF.Gelu, bias=b1_s[:, ffc:ffc + 1],
                                    scale=RSW,
                                )
                    with (
                        tc.tile_pool(name="fc2s", bufs=12) as fs2,
                        tc.tile_pool(name="fco", bufs=4) as fo,
                        tc.tile_pool(name="fc2ps", bufs=1, space="PSUM") as f2ps,
                    ):
                        for db in range(2):
                            y_ps = [f2ps.tile([128, T], F32, tag=f"y{q}", name=f"yps{q}")
                                    for q in range(8)]
                            for f in range(FFPH):
                                if db == 0 and f < 8:
                                    wt = w20[f]
                                else:
                                    wt = fs2.tile([128, 2, 1024], F8, tag="w2")
                                    nc.sync.dma_start(
                                        wt[:],
                                        w2p_v[f, :, :,
                                              db * 1024:(db + 1) * 1024],
                                    )
                                for q in range(8):
                                    nc.tensor.matmul(
                                        y_ps[q][:],
                                        wt[:, :, q * 128:(q + 1) * 128],
                                        gres2[f][:],
                                        start=(f == 0), stop=(f == FFPH - 1),
                                        perf_mode=DR,
                                    )
                            for q in range(8):
                                dc = db * 8 + q
                                # yb = psum * 2^-13 + b2 on the scalar engine
                                yb = fo.tile([128, T], F32, tag="yb")
                                nc.scalar.activation(
                                    yb[:], y_ps[q][:], AF.Identity,
                                    bias=b2_s[:, dc:dc + 1], scale=RSW,
                                )
                                yt = fo.tile([128, T], F32, tag="yt")
                                nc.vector.tensor_tensor(
                                    yt[:], yb[:], x2Ts[dc][:].bitcast(F32),
                                    OP.add,
                                )
                                # two descriptors: a single 256KB transfer
                                # runs on one DMA engine (~11us tail)
                                for th in range(2):
                                    nc.sync.dma_start(
                                        yT_d.ap()[dc * 128:(dc + 1) * 128,
                                                  th * 256:(th + 1) * 256],
                                        yt[:, th * 256:(th + 1) * 256],
                                    )

    nc.compile()
    return nc


_NC_CACHE = None


def _get_nc():
    global _NC_CACHE
    if _NC_CACHE is None:
        m = build()
        m.m = get_hw_module(m.m)
        _NC_CACHE = m
    return _NC_CACHE


def _make_in_maps(x, wq, wk, wv, wo, w1, b1, w2, b2, g1, be1, g2, be2):
    f = lambda a: np.ascontiguousarray(np.asarray(a, dtype=np.float32))
    f8 = lambda a: np.ascontiguousarray(
        np.clip(np.asarray(a, dtype=np.float32) * SW, -240.0, 240.0)
        .astype(ml_dtypes.float8_e4m3)
    )
    x = f(x)
    shared = {
        "wq8": f8(wq), "wk8": f8(wk), "wv8": f8(wv), "wo8": f8(wo),
        "w18": f8(w1), "w28": f8(w2),
        "b1r": np.ascontiguousarray(f(b1).reshape(FFCH, 128).T),
        "b2r": np.ascontiguousarray(f(b2).reshape(NCH, 128).T),
        "g1r": np.ascontiguousarray(f(g1).reshape(NCH, 128).T),
        "be1r": np.ascontiguousarray(f(be1).reshape(NCH, 128).T),
        "g2r": np.ascontiguousarray(f(g2).reshape(NCH, 128).T),
        "be2r": np.ascontiguousarray(f(be2).reshape(NCH, 128).T),
    }
    in_maps = []
    for c in range(N_CORES):
        t0 = c * TB
        xc = np.concatenate([x[0, t0:t0 + TB, :], x[1, t0:t0 + TB, :]], axis=0)
        m = dict(shared)
        m["xT"] = np.ascontiguousarray(xc.T)
        in_maps.append(m)
    return in_maps


def _assemble(results):
    y = np.empty((B, S, D), dtype=np.float32)
    for c in range(N_CORES):
        t0 = c * TB
        yt = results[c]["yT"]
        y[0, t0:t0 + TB, :] = yt[:, 0:TB].T
        y[1, t0:t0 + TB, :] = yt[:, TB:2 * TB].T
    return y


def run(inputs, trace=False, trace_cores=None):
    nc = _get_nc()
    in_maps = _make_in_maps(**inputs)
    res = bass_utils.run_bass_kernel_spmd(
        nc, in_maps, core_ids=list(range(N_CORES)),
        trace=trace, trace_cores=trace_cores,
    )
    return _assemble(res.results), res


def kernel(**inputs):
    y, _ = run(inputs, trace=False)
    return y
